# revision 44
# baseline (speedup 1.0000x reference)
"""DeepEMD Trainium2 kernel: batched 49x49 entropic-OT (Sinkhorn) similarity.

Strategy (8 NeuronCores, data-parallel over batch):
- Host prepack does the cheap O(B*C*N) work exactly in fp32: channel-mean
  centering, unit-normalization of the node vectors, the weight vectors
  w1/w2 = relu(<A, mean(B)>)+1e-3 and the final T/sum(w2) scale. The device
  then only needs the cross Gram Q^T P (= the cosine similarity map), exp,
  and the Sinkhorn iterations.
- Each core gets 128 batches. Host packs, per (chunk j of 128 channels,
  batch b), [Qhat | Phat] (128 x 98) in fp16, grouped 16 batches per DMA
  span; loads are software-pipelined 3 groups ahead on the sync queue.
- PE computes sim_b = Qhat^T Phat per batch: 4 accumulating fp16 matmuls
  (weights = Qhat widened to 128 cols for fast-weight-load; junk rows
  49..127 ignored, moving = Phat 49 cols). 8 batches share one PSUM bank
  ([128, 392]); one vector copy per 8 batches moves rows 0..48 to SBUF.
- A per-batch SBUF->SBUF DMA flattens [49, 49] into row b of a [128, 49*52]
  tile (p padded to 52, pad = -5 so exp(pad) == 0); everything after runs
  batch-on-partitions with full 128-lane DVE, 16-bit where possible.
  Flatten issues are scheduled by engine capacity: gpsimd (SWDGE) streams
  full-rate throughout, scalar's HWDGE is ~3x degraded while load
  descriptors occupy the queues, sync joins after its load issues.
- Sinkhorn runs in the linear domain (K = exp((sim-1)/eps + 16), fp16) with
  Gauss-Seidel updates us = w1/(K vs), vs = w2/(K^T us), ending on the
  u-update (flow row sums == w1, so the final scale is T/sum(w1)). K^T is
  materialized by a second (strided-input) ACT exp so both reduce
  directions are contiguous. Reduces run two-stage: an fp16 2x-mode
  half-add then a half-width reduce. 3.5 iterations give ~4.9e-3
  (tolerance 2e-2).
- logits[b] = T/s1 * us^T ((K.sim_fp16) vs) in a 2x-mode fp16 chain.
"""

import os
import sys

import numpy as np

sys.path.insert(0, "/opt/trn_rl_repo")

import concourse.bass as bass
import concourse.bacc as bacc
import concourse.mybir as mybir
from concourse import tile
from concourse.bass_utils import run_bass_kernel_spmd

B_FULL, C, HW = 1024, 512, 49
HP = HW + 3  # padded inner dim 52: even 26-wide halves for 2x-mode adds
NCORE = 8
BS = B_FULL // NCORE  # 128 batches per core
NCH = C // 128  # 4 chunks of 128 channels (PE contraction dim)
W2 = 2 * HW  # 98 cols per (batch, chunk): [Qhat | Phat]
GRP = 16  # batches per DMA group
NGRP = BS // GRP
JW = GRP * W2  # 1568 cols per chunk-slab in a group
WCH = NCH * JW + 32  # +32 zero pad so the last 128-wide weight view is legal
FWP = HW * HP  # 2548 padded flat row
ITERS = 4
EPS_S = 0.05
TEMP = 12.5 / HW
EXP_BIAS = 16.0  # K = exp((sim-1)/eps + 16): fp16-safe range [2e-4, 1.35]
PAD = -5.0  # sim pad value; exp((PAD-1)/eps+16) == 0 in fp16

f32 = mybir.dt.float32
f16 = mybir.dt.float16
Alu = mybir.AluOpType
Act = mybir.ActivationFunctionType
AxX = mybir.AxisListType.X


def build_nc(debug=False):
    nc = bacc.Bacc(None, target_bir_lowering=False, debug=debug)
    aug = nc.declare_dram_parameter("aug", [NGRP, 128, WCH], f16, isOutput=False)
    auxp = nc.declare_dram_parameter("aux", [BS, HW + HW + 1], f32, isOutput=False)
    outp = nc.declare_dram_parameter("out", [BS, 1], f32, isOutput=True)

    with tile.TileContext(nc) as tc:
        with (
            tc.tile_pool(name="big", bufs=1) as big,
            tc.tile_pool(name="stage", bufs=8) as stg,
            tc.tile_pool(name="gcopy", bufs=16) as gcp,
            tc.tile_pool(name="small", bufs=1) as sml,
            tc.tile_pool(name="psum", bufs=8, space="PSUM") as pp,
        ):
            # software-pipelined loads: 3 groups of lookahead (8 stage bufs, so
            # tiles never wait on reuse), with later loads issued between
            # flatten batches so queue FIFO order interleaves them
            ths = [stg.tile([128, WCH], f16, tag="h", name=f"hg{g}") for g in range(NGRP)]
            # early groups split across many queues (low latency), late groups
            # use big descriptors (bytes-efficient): per-queue work ~30us
            NSPLS = [4, 4, 4, 4, 4, 4, 4, 4]

            def load_group(g):
                nspl = NSPLS[g]
                # chunk-aligned splits: matmuls of chunk j depend only on
                # sub-DMA j (sub-tile range tracking), so compute can start
                # before the whole group lands
                bounds = [0, JW, 2 * JW, 3 * JW, WCH]
                for ss in range(nspl):
                    nc.sync.dma_start(
                        ths[g][:, bounds[ss] : bounds[ss + 1]],
                        aug[g, :, bounds[ss] : bounds[ss + 1]],
                    )

            for g in range(3):
                load_group(g)

            flatG = big.tile([BS, FWP], f16, tag="flatG", name="flatG")
            nc.vector.memset(flatG[:], PAD)
            aux = sml.tile([BS, HW + HW + 1], f32, tag="aux", name="aux")
            w1 = aux[:, 0:HW]
            w2 = aux[:, HW : 2 * HW]
            rs2t = aux[:, 2 * HW : 2 * HW + 1]

            ebias = sml.tile([BS, 1], f32, tag="ebias", name="ebias")
            nc.vector.memset(ebias[:], EXP_BIAS - 1.0 / EPS_S)
            # warm the ACT exp table set early (no data deps -> runs under the
            # phase-1 DMA shadow, hiding the ~2.7us PSEUDO_LOAD_ACT_FUNC_SET)
            wrm = sml.tile([BS, 1], f32, tag="wrm", name="wrm")
            nc.vector.memset(wrm[:], 1.0)
            nc.scalar.activation(wrm[:], wrm[:], Act.Exp)

            # ---------------- Phase 1: cross-Gram + flatten -------------------
            for g in range(NGRP):
                th = ths[g]
                if g + 3 < NGRP:
                    load_group(g + 3)
                if g == 4:
                    # aux isn't needed until iter0; issue it after the load
                    # stream so its 128 descriptors don't delay a load queue
                    nc.sync.dma_start(aux[:], auxp[:, :])
                for half in range(2):
                    ps = pp.tile([128, 8 * HW], f32, tag="gram", name="gram")
                    for b8 in range(8):
                        bb = half * 8 + b8
                        for j in range(NCH):
                            woff = j * JW + bb * W2
                            nc.tensor.matmul(
                                ps[:, b8 * HW : (b8 + 1) * HW],
                                th[:, woff : woff + 128],
                                th[:, woff + HW : woff + W2],
                                start=(j == 0),
                                stop=(j == NCH - 1),
                            )
                    st = gcp.tile([HW, 8 * HW], f16, tag="gs", name="gs")
                    # two 4-batch copies: the first 4 flattens issue while the
                    # second half of the bank is still being copied
                    nc.vector.tensor_copy(st[:, 0 : 4 * HW], ps[0:HW, 0 : 4 * HW])
                    nc.vector.tensor_copy(st[:, 4 * HW :], ps[0:HW, 4 * HW :])
                    for b8 in range(8):
                        b = g * GRP + half * 8 + b8
                        # issuer capacity: gpsimd (SWDGE) runs full-rate
                        # throughout; scalar (HWDGE) is ~3x degraded while
                        # load descriptors stream; sync is free after its
                        # load issues (~g4). Schedule accordingly.
                        if g < 4:
                            dmae = (nc.gpsimd, nc.scalar, nc.gpsimd, nc.gpsimd,
                                    nc.gpsimd, nc.scalar, nc.gpsimd, nc.gpsimd)[b8]
                        elif g == 4:
                            dmae = (nc.sync, nc.gpsimd, nc.sync, nc.gpsimd,
                                    nc.sync, nc.gpsimd, nc.scalar, nc.gpsimd)[b8]
                        else:
                            dmae = (nc.sync, nc.gpsimd, nc.sync, nc.scalar,
                                    nc.sync, nc.gpsimd, nc.sync, nc.scalar)[b8]
                        dst = flatG[b : b + 1, :].rearrange(
                            "b (q c) -> b q c", c=HP
                        )[:, :, 0:HW]
                        dmae.dma_start(dst, st[:, b8 * HW : (b8 + 1) * HW], single_packet=True)

            # ---------------- Phase 2: K = exp, Sinkhorn (fp16) ---------------
            Kb = big.tile([BS, FWP], f16, tag="K", name="K")  # [b, q, p50]
            Kt = big.tile([BS, FWP], f16, tag="Kt", name="Kt")  # [b, p, q50]
            nc.vector.memset(Kt[:], 0.0)
            tb = big.tile([BS, FWP], f16, tag="tb", name="tb")
            tb2 = big.tile([BS, FWP], f16, tag="tb2", name="tb2")

            us = sml.tile([BS, HP], f16, tag="us", name="us")
            vs = sml.tile([BS, HP], f16, tag="vs", name="vs")
            nc.vector.memset(us[:], 0.0)
            nc.vector.memset(vs[:], 0.0)
            kv = sml.tile([BS, HW], f32, tag="kv", name="kv")
            rkv = sml.tile([BS, HW], f32, tag="rkv", name="rkv")
            lg = sml.tile([BS, 1], f32, tag="lg", name="lg")
            lgf = sml.tile([BS, 1], f32, tag="lgf", name="lgf")

            def v3(t):  # [128, 49, 52] view
                return t[:].rearrange("p (q c) -> p q c", c=HP)

            HH = HP // 2  # 26
            thm = big.tile([BS, HW * HH], f16, tag="thm", name="thm")

            def vh(t, lo):  # [128, 49, 26] half view of a [128, 49, 52] tile
                return t[:].rearrange("p (q c) -> p q c", c=HP)[:, :, lo : lo + HH]

            def red(dst, t):
                # dst = sum over p of t[128, 49, 52], via fp16 2x-mode
                # half-add then a half-width reduce
                h3 = thm[:].rearrange("p (q c) -> p q c", c=HH)
                nc.vector.tensor_add(h3, vh(t, 0), vh(t, HH))
                nc.vector.tensor_reduce(dst, h3, axis=AxX, op=Alu.add)

            # K = exp((sim-1)/eps + 16); pad cols exp(-104) flush to 0 in fp16
            nc.scalar.activation(
                Kb[:], flatG[:], Act.Exp, scale=1.0 / EPS_S, bias=ebias[:]
            )
            # K^T via strided-input exp: in [b, p, q] (inner stride 52),
            # out [b, p, q] contiguous rows of 49 (pad q col stays 0)
            simT = flatG[:].rearrange("b (q c) -> b c q", c=HP)[:, 0:HW, :]
            KtV = v3(Kt)[:, :, 0:HW]
            nc.scalar.activation(KtV, simT, Act.Exp, scale=1.0 / EPS_S, bias=ebias[:])

            bus = us[:].unsqueeze(1).broadcast_to([BS, HW, HP])
            bvs = vs[:].unsqueeze(1).broadcast_to([BS, HW, HP])
            # ends on the u-update: flow row sums = w1 exactly, so the final
            # scale is T/sum(w1) (host-sent)
            for it in range(ITERS):
                if it == 0:
                    red(kv[:], Kb)
                else:
                    nc.vector.tensor_mul(v3(tb), v3(Kb), bvs)
                    red(kv[:], tb)
                nc.vector.reciprocal_approx_fast(rkv[:], kv[:])
                nc.vector.tensor_mul(us[:, 0:HW], w1, rkv[:])
                if it == ITERS - 1:
                    break
                nc.vector.tensor_mul(v3(tb2), v3(Kt), bus)
                red(kv[:], tb2)
                nc.vector.reciprocal_approx_fast(rkv[:], kv[:])
                nc.vector.tensor_mul(vs[:, 0:HW], w2, rkv[:])

            # ---------------- Phase 3: logits ---------------------------------
            # logits = sum_q us[q] * sum_p (K*vs)[q,p] * sim[q,p], all fp16 2x
            # tb still holds Kb*bvs from the last iteration's u-side (vs is
            # unchanged after ending on the u-update) -- reuse it directly;
            # flatG is already fp16 so it feeds the 2x-mode mul as-is
            nc.vector.tensor_mul(v3(tb2), v3(tb), v3(flatG))
            red(kv[:], tb2)
            nc.vector.tensor_mul(rkv[:], kv[:], us[:, 0:HW])
            nc.vector.tensor_reduce(lg[:], rkv[:], axis=AxX, op=Alu.add)
            # write logits via a 32x32 stream transpose so the out DMA is 4
            # single-descriptor transfers instead of 128 per-partition ones
            lg32 = sml.tile([BS, 32], f32, tag="lg32", name="lg32")
            t32 = sml.tile([BS, 32], f32, tag="t32", name="t32")
            nc.vector.tensor_mul(lg32[:, 0:1], lg[:], rs2t)
            nc.vector.transpose(t32[:], lg32[:])
            for k in range(4):
                nc.sync.dma_start(
                    outp[32 * k : 32 * (k + 1), :], t32[32 * k : 32 * k + 1, 0:32]
                )


    nc.compile()
    return nc


_NC = None


def _get_nc():
    global _NC
    if _NC is None:
        _NC = build_nc()
    return _NC


def _prep_in_maps(feature_map1, feature_map2):
    q = np.ascontiguousarray(np.asarray(feature_map1, dtype=np.float32)).reshape(
        B_FULL, C, HW
    )
    p = np.ascontiguousarray(np.asarray(feature_map2, dtype=np.float32)).reshape(
        B_FULL, C, HW
    )
    # weight vectors from the RAW features (exact, fp32)
    w1 = np.maximum((q * p.mean(axis=2, keepdims=True)).sum(axis=1), 0.0) + 0.001
    w2 = np.maximum((p * q.mean(axis=2, keepdims=True)).sum(axis=1), 0.0) + 0.001
    rs2t = (TEMP / w1.sum(axis=1, keepdims=True)).astype(np.float32)
    # center + unit-normalize the node vectors (exact, fp32), then fp16
    qc = q - q.mean(axis=1, keepdims=True)
    pc = p - p.mean(axis=1, keepdims=True)
    qn = (qc / (np.linalg.norm(qc, axis=1, keepdims=True) + 1e-8)).astype(np.float16)
    pn = (pc / (np.linalg.norm(pc, axis=1, keepdims=True) + 1e-8)).astype(np.float16)

    in_maps = []
    for i in range(NCORE):
        sl = slice(i * BS, (i + 1) * BS)
        # [NGRP, 128ch, NCH, GRP, 98] -> contiguous group spans
        a = np.zeros((NGRP, 128, WCH), np.float16)
        qi = qn[sl].reshape(NGRP, GRP, NCH, 128, HW).transpose(0, 3, 2, 1, 4)
        pi = pn[sl].reshape(NGRP, GRP, NCH, 128, HW).transpose(0, 3, 2, 1, 4)
        blk = np.empty((NGRP, 128, NCH, GRP, W2), np.float16)
        blk[..., 0:HW] = qi
        blk[..., HW:W2] = pi
        a[:, :, : NCH * JW] = blk.reshape(NGRP, 128, NCH * JW)
        aux = np.empty((BS, HW + HW + 1), np.float32)
        aux[:, 0:HW] = w1[sl]
        aux[:, HW : 2 * HW] = w2[sl]
        aux[:, 2 * HW :] = rs2t[sl]
        in_maps.append({"aug": a, "aux": aux})
    return in_maps


def run(feature_map1, feature_map2, trace=False):
    in_maps = _prep_in_maps(feature_map1, feature_map2)
    nc = _get_nc()
    res = run_bass_kernel_spmd(nc, in_maps, core_ids=list(range(NCORE)), trace=trace)
    out = np.concatenate(
        [np.asarray(res.results[i]["out"]).reshape(BS) for i in range(NCORE)]
    ).astype(np.float32)
    return out, res


def kernel(feature_map1, feature_map2):
    out, _ = run(feature_map1, feature_map2, trace=False)
    return out


# revision 45
# speedup vs baseline: 1.1417x; 1.1417x over previous
"""DeepEMD Trainium2 kernel: batched 49x49 entropic-OT (Sinkhorn) similarity.

Strategy (8 NeuronCores, data-parallel over batch):
- Host prepack does the cheap O(B*C*N) work exactly in fp32: channel-mean
  centering, unit-normalization of the node vectors, the weight vectors
  w1/w2 = relu(<A, mean(B)>)+1e-3 and the final T/sum(w2) scale. The device
  then only needs the cross Gram Q^T P (= the cosine similarity map), exp,
  and the Sinkhorn iterations.
- Each core gets 128 batches. Host packs, per (chunk j of 128 channels,
  batch b), [Qhat | Phat] (128 x 98) in fp16, grouped 16 batches per DMA
  span; loads are software-pipelined 3 groups ahead on the sync queue.
- PE computes sim_b = Qhat^T Phat per batch: 4 accumulating fp16 matmuls
  (weights = Qhat widened to 128 cols for fast-weight-load; junk rows
  49..127 ignored, moving = Phat 49 cols). 8 batches share one PSUM bank
  ([128, 392]); one vector copy per 8 batches moves rows 0..48 to SBUF.
- A per-batch SBUF->SBUF DMA flattens [49, 49] into row b of a [128, 49*52]
  tile (p padded to 52, pad = -5 so exp(pad) == 0); everything after runs
  batch-on-partitions with full 128-lane DVE, 16-bit where possible.
  Flatten issues are scheduled by engine capacity: gpsimd (SWDGE) streams
  full-rate throughout, scalar's HWDGE is ~3x degraded while load
  descriptors occupy the queues, sync joins after its load issues.
- Sinkhorn runs in the linear domain (K = exp((sim-1)/eps + 16), fp16) with
  Gauss-Seidel updates us = w1/(K vs), vs = w2/(K^T us), ending on the
  u-update (flow row sums == w1, so the final scale is T/sum(w1)). K^T is
  materialized by a second (strided-input) ACT exp so both reduce
  directions are contiguous. Reduces run two-stage: an fp16 2x-mode
  half-add then a half-width reduce. 3.5 iterations give ~4.9e-3
  (tolerance 2e-2).
- logits[b] = T/s1 * us^T ((K.sim_fp16) vs) in a 2x-mode fp16 chain.
"""

import os
import sys

import numpy as np

sys.path.insert(0, "/opt/trn_rl_repo")

import concourse.bass as bass
import concourse.bacc as bacc
import concourse.mybir as mybir
from concourse import tile
from concourse.bass_utils import run_bass_kernel_spmd

B_FULL, C, HW = 1024, 512, 49
HP = HW + 3  # padded inner dim 52: even 26-wide halves for 2x-mode adds
NCORE = 8
BS = B_FULL // NCORE  # 128 batches per core
NCH = C // 128  # 4 chunks of 128 channels (PE contraction dim)
W2 = 2 * HW  # 98 cols per (batch, chunk): [Qhat | Phat]
GRP = 16  # batches per DMA group
NGRP = BS // GRP
JW = GRP * W2  # 1568 cols per chunk-slab in a group
WCH = NCH * JW + 32  # +32 zero pad so the last 128-wide weight view is legal
FWP = HW * HP  # 2548 padded flat row
ITERS = 4
EPS_S = 0.05
TEMP = 12.5 / HW
EXP_BIAS = 16.0  # K = exp((sim-1)/eps + 16): fp16-safe range [2e-4, 1.35]
PAD = -5.0  # sim pad value; exp((PAD-1)/eps+16) == 0 in fp16

f32 = mybir.dt.float32
f16 = mybir.dt.float16
Alu = mybir.AluOpType
Act = mybir.ActivationFunctionType
AxX = mybir.AxisListType.X


def build_nc(debug=False):
    nc = bacc.Bacc(None, target_bir_lowering=False, debug=debug)
    aug = nc.declare_dram_parameter("aug", [NGRP, 128, WCH], f16, isOutput=False)
    auxp = nc.declare_dram_parameter("aux", [BS, HW + HW + 1], f32, isOutput=False)
    outp = nc.declare_dram_parameter("out", [BS, 1], f32, isOutput=True)

    with tile.TileContext(nc) as tc:
        with (
            tc.tile_pool(name="big", bufs=1) as big,
            tc.tile_pool(name="stage", bufs=8) as stg,
            tc.tile_pool(name="gcopy", bufs=16) as gcp,
            tc.tile_pool(name="small", bufs=1) as sml,
            tc.tile_pool(name="psum", bufs=8, space="PSUM") as pp,
        ):
            # software-pipelined loads: 3 groups of lookahead (8 stage bufs, so
            # tiles never wait on reuse), with later loads issued between
            # flatten batches so queue FIFO order interleaves them
            ths = [stg.tile([128, WCH], f16, tag="h", name=f"hg{g}") for g in range(NGRP)]
            # early groups split across many queues (low latency), late groups
            # use big descriptors (bytes-efficient): per-queue work ~30us
            NSPLS = [4, 4, 4, 4, 4, 4, 4, 4]

            def load_group(g):
                nspl = NSPLS[g]
                # chunk-aligned splits: matmuls of chunk j depend only on
                # sub-DMA j (sub-tile range tracking), so compute can start
                # before the whole group lands
                bounds = [0, JW, 2 * JW, 3 * JW, WCH]
                for ss in range(nspl):
                    nc.sync.dma_start(
                        ths[g][:, bounds[ss] : bounds[ss + 1]],
                        aug[g, :, bounds[ss] : bounds[ss + 1]],
                    )

            for g in range(3):
                load_group(g)

            flatG = big.tile([BS, FWP], f16, tag="flatG", name="flatG")
            nc.vector.memset(flatG[:], PAD)
            aux = sml.tile([BS, HW + HW + 1], f32, tag="aux", name="aux")
            w1 = aux[:, 0:HW]
            w2 = aux[:, HW : 2 * HW]
            rs2t = aux[:, 2 * HW : 2 * HW + 1]

            ebias = sml.tile([BS, 1], f32, tag="ebias", name="ebias")
            nc.vector.memset(ebias[:], EXP_BIAS - 1.0 / EPS_S)
            # warm the ACT exp table set early (no data deps -> runs under the
            # phase-1 DMA shadow, hiding the ~2.7us PSEUDO_LOAD_ACT_FUNC_SET)
            wrm = sml.tile([BS, 1], f32, tag="wrm", name="wrm")
            nc.vector.memset(wrm[:], 1.0)
            nc.scalar.activation(wrm[:], wrm[:], Act.Exp)

            # ---------------- Phase 1: cross-Gram + flatten -------------------
            for g in range(NGRP):
                th = ths[g]
                if g + 3 < NGRP:
                    load_group(g + 3)
                if g == 4:
                    # aux isn't needed until iter0; issue it after the load
                    # stream so its 128 descriptors don't delay a load queue
                    nc.sync.dma_start(aux[:], auxp[:, :])
                for half in range(2):
                    ps = pp.tile([128, 8 * HW], f32, tag="gram", name="gram")
                    for b8 in range(8):
                        bb = half * 8 + b8
                        for j in range(NCH):
                            woff = j * JW + bb * W2
                            nc.tensor.matmul(
                                ps[:, b8 * HW : (b8 + 1) * HW],
                                th[:, woff : woff + 128],
                                th[:, woff + HW : woff + W2],
                                start=(j == 0),
                                stop=(j == NCH - 1),
                            )
                    st = gcp.tile([HW, 8 * HW], f16, tag="gs", name="gs")
                    nc.vector.tensor_copy(st[:], ps[0:HW, :])
                    for b8 in range(8):
                        b = g * GRP + half * 8 + b8
                        # issuer capacity: gpsimd (SWDGE) runs full-rate
                        # throughout; scalar (HWDGE) is ~3x degraded while
                        # load descriptors stream; sync is free after its
                        # load issues (~g4). Schedule accordingly.
                        if g < 4:
                            dmae = (nc.gpsimd, nc.scalar, nc.gpsimd, nc.gpsimd,
                                    nc.gpsimd, nc.scalar, nc.gpsimd, nc.gpsimd)[b8]
                        elif g == 4:
                            dmae = (nc.sync, nc.gpsimd, nc.sync, nc.gpsimd,
                                    nc.sync, nc.gpsimd, nc.scalar, nc.gpsimd)[b8]
                        else:
                            dmae = (nc.sync, nc.gpsimd, nc.sync, nc.scalar,
                                    nc.sync, nc.gpsimd, nc.sync, nc.scalar)[b8]
                        dst = flatG[b : b + 1, :].rearrange(
                            "b (q c) -> b q c", c=HP
                        )[:, :, 0:HW]
                        dmae.dma_start(dst, st[:, b8 * HW : (b8 + 1) * HW], single_packet=True)

            # ---------------- Phase 2: K = exp, Sinkhorn (fp16) ---------------
            Kb = big.tile([BS, FWP], f16, tag="K", name="K")  # [b, q, p50]
            Kt = big.tile([BS, FWP], f16, tag="Kt", name="Kt")  # [b, p, q50]
            nc.vector.memset(Kt[:], 0.0)
            tb = big.tile([BS, FWP], f16, tag="tb", name="tb")
            tb2 = big.tile([BS, FWP], f16, tag="tb2", name="tb2")

            us = sml.tile([BS, HP], f16, tag="us", name="us")
            vs = sml.tile([BS, HP], f16, tag="vs", name="vs")
            nc.vector.memset(us[:], 0.0)
            nc.vector.memset(vs[:], 0.0)
            kv = sml.tile([BS, HW], f32, tag="kv", name="kv")
            rkv = sml.tile([BS, HW], f32, tag="rkv", name="rkv")
            lg = sml.tile([BS, 1], f32, tag="lg", name="lg")
            lgf = sml.tile([BS, 1], f32, tag="lgf", name="lgf")

            def v3(t):  # [128, 49, 52] view
                return t[:].rearrange("p (q c) -> p q c", c=HP)

            HH = HP // 2  # 26
            thm = big.tile([BS, HW * HH], f16, tag="thm", name="thm")

            def vh(t, lo):  # [128, 49, 26] half view of a [128, 49, 52] tile
                return t[:].rearrange("p (q c) -> p q c", c=HP)[:, :, lo : lo + HH]

            def red(dst, t):
                # dst = sum over p of t[128, 49, 52], via fp16 2x-mode
                # half-add then a half-width reduce
                h3 = thm[:].rearrange("p (q c) -> p q c", c=HH)
                nc.vector.tensor_add(h3, vh(t, 0), vh(t, HH))
                nc.vector.tensor_reduce(dst, h3, axis=AxX, op=Alu.add)

            # K = exp((sim-1)/eps + 16); pad cols exp(-104) flush to 0 in fp16
            nc.scalar.activation(
                Kb[:], flatG[:], Act.Exp, scale=1.0 / EPS_S, bias=ebias[:]
            )
            # K^T via strided-input exp: in [b, p, q] (inner stride 52),
            # out [b, p, q] contiguous rows of 49 (pad q col stays 0)
            simT = flatG[:].rearrange("b (q c) -> b c q", c=HP)[:, 0:HW, :]
            KtV = v3(Kt)[:, :, 0:HW]
            nc.scalar.activation(KtV, simT, Act.Exp, scale=1.0 / EPS_S, bias=ebias[:])

            bus = us[:].unsqueeze(1).broadcast_to([BS, HW, HP])
            bvs = vs[:].unsqueeze(1).broadcast_to([BS, HW, HP])
            # ends on the u-update: flow row sums = w1 exactly, so the final
            # scale is T/sum(w1) (host-sent)
            for it in range(ITERS):
                if it == 0:
                    red(kv[:], Kb)
                else:
                    nc.vector.tensor_mul(v3(tb), v3(Kb), bvs)
                    red(kv[:], tb)
                nc.vector.reciprocal_approx_fast(rkv[:], kv[:])
                nc.vector.tensor_mul(us[:, 0:HW], w1, rkv[:])
                if it == ITERS - 1:
                    break
                nc.vector.tensor_mul(v3(tb2), v3(Kt), bus)
                red(kv[:], tb2)
                nc.vector.reciprocal_approx_fast(rkv[:], kv[:])
                nc.vector.tensor_mul(vs[:, 0:HW], w2, rkv[:])

            # ---------------- Phase 3: logits ---------------------------------
            # logits = sum_q us[q] * sum_p (K*vs)[q,p] * sim[q,p], all fp16 2x
            # tb still holds Kb*bvs from the last iteration's u-side (vs is
            # unchanged after ending on the u-update) -- reuse it directly;
            # flatG is already fp16 so it feeds the 2x-mode mul as-is
            nc.vector.tensor_mul(v3(tb2), v3(tb), v3(flatG))
            red(kv[:], tb2)
            nc.vector.tensor_mul(rkv[:], kv[:], us[:, 0:HW])
            nc.vector.tensor_reduce(lg[:], rkv[:], axis=AxX, op=Alu.add)
            # write logits via a 32x32 stream transpose so the out DMA is 4
            # single-descriptor transfers instead of 128 per-partition ones
            lg32 = sml.tile([BS, 32], f32, tag="lg32", name="lg32")
            t32 = sml.tile([BS, 32], f32, tag="t32", name="t32")
            nc.vector.tensor_mul(lg32[:, 0:1], lg[:], rs2t)
            nc.vector.transpose(t32[:], lg32[:])
            for k in range(4):
                nc.sync.dma_start(
                    outp[32 * k : 32 * (k + 1), :], t32[32 * k : 32 * k + 1, 0:32]
                )


    nc.compile()
    return nc


_NC = None


def _get_nc():
    global _NC
    if _NC is None:
        _NC = build_nc()
    return _NC


def _prep_in_maps(feature_map1, feature_map2):
    q = np.ascontiguousarray(np.asarray(feature_map1, dtype=np.float32)).reshape(
        B_FULL, C, HW
    )
    p = np.ascontiguousarray(np.asarray(feature_map2, dtype=np.float32)).reshape(
        B_FULL, C, HW
    )
    # weight vectors from the RAW features (exact, fp32)
    w1 = np.maximum((q * p.mean(axis=2, keepdims=True)).sum(axis=1), 0.0) + 0.001
    w2 = np.maximum((p * q.mean(axis=2, keepdims=True)).sum(axis=1), 0.0) + 0.001
    rs2t = (TEMP / w1.sum(axis=1, keepdims=True)).astype(np.float32)
    # center + unit-normalize the node vectors (exact, fp32), then fp16
    qc = q - q.mean(axis=1, keepdims=True)
    pc = p - p.mean(axis=1, keepdims=True)
    qn = (qc / (np.linalg.norm(qc, axis=1, keepdims=True) + 1e-8)).astype(np.float16)
    pn = (pc / (np.linalg.norm(pc, axis=1, keepdims=True) + 1e-8)).astype(np.float16)

    in_maps = []
    for i in range(NCORE):
        sl = slice(i * BS, (i + 1) * BS)
        # [NGRP, 128ch, NCH, GRP, 98] -> contiguous group spans
        a = np.zeros((NGRP, 128, WCH), np.float16)
        qi = qn[sl].reshape(NGRP, GRP, NCH, 128, HW).transpose(0, 3, 2, 1, 4)
        pi = pn[sl].reshape(NGRP, GRP, NCH, 128, HW).transpose(0, 3, 2, 1, 4)
        blk = np.empty((NGRP, 128, NCH, GRP, W2), np.float16)
        blk[..., 0:HW] = qi
        blk[..., HW:W2] = pi
        a[:, :, : NCH * JW] = blk.reshape(NGRP, 128, NCH * JW)
        aux = np.empty((BS, HW + HW + 1), np.float32)
        aux[:, 0:HW] = w1[sl]
        aux[:, HW : 2 * HW] = w2[sl]
        aux[:, 2 * HW :] = rs2t[sl]
        in_maps.append({"aug": a, "aux": aux})
    return in_maps


def run(feature_map1, feature_map2, trace=False):
    in_maps = _prep_in_maps(feature_map1, feature_map2)
    nc = _get_nc()
    res = run_bass_kernel_spmd(nc, in_maps, core_ids=list(range(NCORE)), trace=trace)
    out = np.concatenate(
        [np.asarray(res.results[i]["out"]).reshape(BS) for i in range(NCORE)]
    ).astype(np.float32)
    return out, res


def kernel(feature_map1, feature_map2):
    out, _ = run(feature_map1, feature_map2, trace=False)
    return out
